# revision 35
# baseline (speedup 1.0000x reference)
"""MultiHeadClassifier (MoE routing) Trainium2 kernel.

Problem: B=65536 samples of dim D=1024, each routed by task_id to one of
T=16 two-layer heads (D->H=128 relu -> C=10). The dense reference computes
all 16 heads for every sample (275 GFLOP); here we route on the host and
compute only each sample's own head (~17 GFLOP), data-parallel with 2 task
slots per NeuronCore across 8 cores.

Strategy (v2, fp8-e3m4 moving operand):
  - Host: bucket samples by task. Tasks ranked by count; core c owns
    rank[c] (slot 0) and rank[15-c] (slot 1), so slot 0 is sized for the
    largest task and slot 1 for the 9th-largest: M0 + M1 < 2*max pad.
    Pad rows point at row 0 (results discarded on unshard).
  - x is quantized host-side to float8 e3m4 (4 mantissa bits; x~N(0,1)
    fits the +-15.5 range). Moving-operand DMA halves vs bf16; the PE
    upcasts e3m4 through the same e10m11 path as bf16 (1 col/cycle), so
    only the mantissa below bit 4 is lost. W1/W2 stay bf16 (their DMA is
    tiny); mixed-dtype matmul (bf16 stationary x e3m4 moving) is legal on
    TRN2. Measured end-to-end rel err ~1.4e-2 (vs 3.4e-3 all-bf16).
  - Host layout xAll [DC, 128, M]: chunk-major so each per-(block, dc)
    DMA reads 128 rows of XB contiguous bytes.
  - Device per (slot, m-tile of 512): 8 accumulating matmuls (W1 d-chunk
    [128,128] bf16 stationary, e3m4 x chunk [128,512] moving) -> PSUM;
    ScalarE fused bias+ReLU -> bf16 h; one matmul with W2 [128,10] ->
    PSUM [10,512]; DVE fused bias-add -> bf16 out tile; DMA out.
  - Host scatters per-task outputs back to the original order (f32).
"""

import sys

import numpy as np

for _p in ("/opt/trn_rl_repo", "/root/.axon_site/_ro/trn_rl_repo"):
    if _p not in sys.path:
        sys.path.append(_p)

import concourse.bacc as bacc
import concourse.mybir as mybir
from concourse.bass_utils import run_bass_kernel_spmd
from concourse.tile import TileContext

B, D, T, H, C = 65536, 1024, 16, 128, 10
N_CORES = 8
S = T // N_CORES  # task slots per core = 2
DC = D // 128  # d-chunks of 128 = 8
MT = 512  # m-tile (PSUM bank = 512 f32)
XB = 4096  # x DMA block (columns)
X_BUFS = 4
N_WARMUP = 12

_F32 = mybir.dt.float32
_BF16 = mybir.dt.bfloat16
_E3M4 = mybir.dt.float8e3

MM_DTYPE = "e3"  # kept for test.py compat


def _np_bf16():
    import ml_dtypes

    return np.dtype(ml_dtypes.bfloat16)


def _np_e3m4():
    import ml_dtypes

    return np.dtype(ml_dtypes.float8_e3m4)


def _chunks(total, step):
    out = []
    p = 0
    while p < total:
        c = min(step, total - p)
        out.append((p, c))
        p += c
    return out


def _blocks(total, step, ramp=False):
    """Like _chunks but ends with a small (<=512) final block so the
    compute tail after the last DMA is short. With ramp=True the first
    blocks are 512, 1024 so early compute is never starved waiting for
    a whole multi-MB block to stream in."""
    out = []
    p = 0
    rem = total
    if ramp:
        # halve the first block so the PE's first full-accumulation
        # subtile isn't gated on a whole 4MB block streaming in. Chunks
        # stay >=256KB: still transfer-bound, not issue-bound.
        for c in (XB // 2,):
            if rem <= c + 512:
                break
            out.append((p, c))
            p += c
            rem -= c
    while rem > 0:
        if rem <= 512 or rem <= step:
            c = rem
        elif rem <= step + 512:
            c = rem - 512
        else:
            c = step
        out.append((p, c))
        p += c
        rem -= c
    return out


def _build(M_slots):
    """M_slots: per-slot padded column counts (M0, M1)."""
    M = sum(M_slots)
    nc = bacc.Bacc(None, target_bir_lowering=False)
    xAll = nc.declare_dram_parameter("xAll", [DC, 128, M], _E3M4, isOutput=False)
    # w1 arrives host-repacked as [S, 128, DC*H]: partition-major, 2KB/row
    w1 = nc.declare_dram_parameter("w1", [S, 128, DC * H], _BF16, isOutput=False)
    b1 = nc.declare_dram_parameter("b1", [S, H], _F32, isOutput=False)
    w2 = nc.declare_dram_parameter("w2", [S, H, C], _BF16, isOutput=False)
    b2 = nc.declare_dram_parameter("b2", [S, C], _F32, isOutput=False)
    outT = nc.declare_dram_parameter("outT", [C, M], _BF16, isOutput=True)

    relu = mybir.ActivationFunctionType.Relu

    with TileContext(nc) as tc:
        with (
            tc.tile_pool(name="wpool", bufs=2) as wpool,
            tc.tile_pool(name="xpool", bufs=X_BUFS) as xpool,
            tc.tile_pool(name="hpool", bufs=8) as hpool,
            tc.tile_pool(name="opool", bufs=2) as opool,
            tc.tile_pool(name="warm", bufs=1) as warm,
            tc.tile_pool(name="psum1", bufs=5, space="PSUM") as psum1,
            tc.tile_pool(name="psum2", bufs=2, space="PSUM") as psum2,
            tc.tile_pool(name="psumw", bufs=1, space="PSUM") as psumw,
        ):  # PSUM banks: 5 + 2 + 1 = 8
            # PE warmup: dummy matmuls release the HAM clock-gate (~3.4us of
            # sustained PE busy) while the first x block streams in.
            wsrc = warm.tile([128, 256], _F32, tag="wsrc")
            nc.gpsimd.memset(wsrc[:], 0.0)
            wv_w = wsrc[:].bitcast(_BF16)  # [128, 512]
            wv_x = wsrc[:].bitcast(_E3M4)  # [128, 1024]
            wps = psumw.tile([128, MT], _F32, tag="wps")
            for _ in range(N_WARMUP):
                nc.tensor.matmul(
                    wps[:], wv_w[:, :128], wv_x[:, :MT], start=True, stop=True
                )
            # weight loads ride the scalar HWDGE ring. Slot 1 (processed
            # first) is hoisted to kernel start; slot 0's loads are emitted
            # after the first block's chunk DMAs (its compute starts ~15us
            # in) so they don't delay the first x chunks on this ring.
            wts = [None] * S

            def load_weights(s):
                w1t = wpool.tile([128, DC, H], _BF16, tag="w1", name=f"w1t{s}")
                nc.scalar.dma_start(
                    w1t, w1[s].rearrange("p (dc h) -> p dc h", dc=DC)
                )
                b1t = wpool.tile([H, 1], _F32, tag="b1", name=f"b1t{s}")
                nc.scalar.dma_start(b1t, b1[s][:, None])
                w2t = wpool.tile([H, C], _BF16, tag="w2", name=f"w2t{s}")
                nc.scalar.dma_start(w2t, w2[s])
                b2t = wpool.tile([C, 1], _F32, tag="b2", name=f"b2t{s}")
                nc.scalar.dma_start(b2t, b2[s][:, None])
                wts[s] = (w1t, b1t, w2t, b2t)

            load_weights(1)
            load_weights(0)

            # global block list: slot 1 first (with ramped block sizes) so
            # the final block is slot 0's small (<=512) tail and the
            # compute/out-DMA tail is short
            blist = []
            offs = [0, M_slots[0]]
            # first-processed slot uses half-size blocks: 256KB chunk DMAs
            # land every ~0.8us, matching the wave's ~0.86us/chunk consumption
            # so the PE ramp is never delivery-starved (full 4MB blocks take
            # ~12us to stream, leaving the PE idle-cold at the start)
            for s in (1, 0):
                for x0, xl in _blocks(M_slots[s], XB // 2 if s == 1 else XB):
                    blist.append((s, offs[s] + x0, xl))

            # Hybrid schedule per block:
            #   - first <=4 subtiles run WAVE-style (dc-outer over 4 psum
            #     banks) so the PE consumes each x chunk ~860ns after it
            #     lands instead of idling until chunk 7 of the block is in
            #     (chunk delivery ~1.6us each, a full block ~12us);
            #   - remaining subtiles run subtile-major (dc-inner into one
            #     bank, drain to ACT immediately).
            # Layer-2 matmuls + bias-adds are deferred on a queue and
            # flushed in PAIRS after later L1 groups: the ACT latency hides
            # under the next 8 matmuls, and back-to-back MM2s share the W2
            # stationary so only the first pays the weight-switch bubble.
            pendq = []  # (ht, ot, m0, mt, w2t, b2t, out_dma or None)

            def flush_one():
                ht, ot_p, m0, mt, w2t_p, b2t_p, out_args = pendq.pop(0)
                ps2 = psum2.tile([C, MT], _F32, tag="ps2")
                nc.tensor.matmul(
                    ps2[:, :mt], w2t_p, ht[:, :mt], start=True, stop=True
                )
                nc.vector.tensor_tensor(
                    ot_p[:, m0 : m0 + mt],
                    ps2[:, :mt],
                    b2t_p.to_broadcast([C, mt]),
                    mybir.AluOpType.add,
                )
                if out_args is not None:
                    eng, o0, ol, otb, src0 = out_args
                    eng.dma_start(outT[:, o0 : o0 + ol], otb[:, src0 : src0 + ol])

            for bi, (s, x0, xl) in enumerate(blist):
                if wts[s] is None:
                    load_weights(s)
                w1t, b1t, w2t, b2t = wts[s]
                # per-d-chunk tiles/DMAs: contiguous rows AND chunk-granular
                # deps, all on the SP HWDGE ring. (Splitting across rings
                # measured WORSE: scalar serializes against ACTIVATEs,
                # gpsimd's software descriptor generation is too slow for
                # 128-descriptor strided reads.)
                xts = []
                for dc in range(DC):
                    xtc = xpool.tile([128, XB], _E3M4, tag=f"x{dc}", name=f"x_{dc}")
                    nc.sync.dma_start(xtc[:, :xl], xAll[dc, :, x0 : x0 + xl])
                    xts.append(xtc)
                ot = opool.tile([C, XB], _BF16, tag="o")
                last_block = bi == len(blist) - 1
                # final block: 256-col subtiles + per-subtile out-DMA on the
                # (by then idle) sync HWDGE ring -> short kernel tail
                subs = _chunks(xl, 256 if last_block else MT)

                def mk_out(j, m0, mt):
                    if last_block:
                        return (nc.sync, x0 + m0, mt, ot, m0)
                    if j == len(subs) - 1:
                        # gpsimd (SWDGE): keeps the waiting out-DMA off the
                        # HWDGE rings so it can't block x-chunk DMAs
                        return (nc.gpsimd, x0, xl, ot, 0)
                    return None

                # drain ALL deferred L2 work before starting this block's
                # wave (every pending ACT is >=1 L1 group old, so no stall):
                # a short final block otherwise leaves the previous block's
                # out-DMA queued until the very end of the kernel
                while pendq:
                    flush_one()
                wave = subs[: min(4, len(subs))]
                rest = subs[len(wave) :]
                ps1s = [
                    psum1.tile([H, MT], _F32, tag="ps1", name=f"ps1_{j}")
                    for j in range(len(wave))
                ]
                for dc in range(DC):
                    for j, (m0, mt) in enumerate(wave):
                        nc.tensor.matmul(
                            ps1s[j][:, :mt],
                            w1t[:, dc, :],
                            xts[dc][:, m0 : m0 + mt],
                            start=(dc == 0),
                            stop=(dc == DC - 1),
                        )
                for j, (m0, mt) in enumerate(wave):
                    ht = hpool.tile([H, MT], _BF16, tag="h")
                    nc.scalar.activation(ht[:, :mt], ps1s[j][:, :mt], relu, bias=b1t)
                    pendq.append((ht, ot, m0, mt, w2t, b2t, mk_out(j, m0, mt)))
                for j0, (m0, mt) in enumerate(rest):
                    j = len(wave) + j0
                    ps1 = psum1.tile([H, MT], _F32, tag="ps1")
                    for dc in range(DC):
                        nc.tensor.matmul(
                            ps1[:, :mt],
                            w1t[:, dc, :],
                            xts[dc][:, m0 : m0 + mt],
                            start=(dc == 0),
                            stop=(dc == DC - 1),
                        )
                    for _ in range(min(2, len(pendq))):
                        flush_one()
                    ht = hpool.tile([H, MT], _BF16, tag="h")
                    nc.scalar.activation(ht[:, :mt], ps1[:, :mt], relu, bias=b1t)
                    pendq.append((ht, ot, m0, mt, w2t, b2t, mk_out(j, m0, mt)))
            while pendq:
                flush_one()
    nc.compile()
    return nc


def _prepare(x, task_id, W1, b1, W2, b2, mm_dtype=MM_DTYPE):
    """Host-side routing + quantization.

    Returns (in_maps, meta) where meta = (slot_tasks, idx, counts, M_slots).
    slot_tasks[s][c] = task owned by core c's slot s.
    """
    np_bf16 = _np_bf16()
    np_e3 = _np_e3m4()
    x = np.ascontiguousarray(np.asarray(x, dtype=np.float32))
    task_id = np.asarray(task_id).astype(np.int64)
    W1 = np.asarray(W1, dtype=np.float32)
    b1 = np.asarray(b1, dtype=np.float32)
    W2 = np.asarray(W2, dtype=np.float32)
    b2 = np.asarray(b2, dtype=np.float32)

    order = np.argsort(task_id, kind="stable")
    counts = np.bincount(task_id, minlength=T)
    starts = np.concatenate([[0], np.cumsum(counts)])

    # rank tasks by count desc; core c gets rank c (slot 0) and rank
    # 15-c (slot 1) so each slot's pad target is its own worst case
    ranks = np.argsort(-counts, kind="stable")
    slot_tasks = [
        [int(ranks[c]) for c in range(N_CORES)],
        [int(ranks[T - 1 - c]) for c in range(N_CORES)],
    ]
    c128 = lambda n: max(128, int(-(-int(n) // 128) * 128))
    M_slots = (
        c128(counts[ranks[0]]),
        c128(counts[ranks[N_CORES]]),
    )

    # idx[s][c] = sample rows for that slot's task, padded with row 0
    idx = [np.zeros((N_CORES, M_slots[s]), dtype=np.int64) for s in range(S)]
    for s in range(S):
        for c in range(N_CORES):
            t = slot_tasks[s][c]
            idx[s][c, : counts[t]] = order[starts[t] : starts[t + 1]]

    xq = x.astype(np_e3)  # RNE quantization; |x| << 15.5 so no overflow
    w1b = W1.astype(np_bf16)
    w2b = W2.astype(np_bf16)

    in_maps = []
    for c in range(N_CORES):
        ts_c = [slot_tasks[s][c] for s in range(S)]
        rows = np.concatenate([idx[s][c] for s in range(S)])  # [M]
        xg = xq[rows]  # [M, D] e3m4
        # chunk-major [DC, 128, M]: row (dc, p) holds x[:, dc*128+p]
        xT = np.ascontiguousarray(
            xg.reshape(-1, DC, 128).transpose(1, 2, 0)
        )
        # repack W1 [D, H] -> [128, DC*H] (partition-major, 2KB DMA rows)
        w1p = (
            w1b[ts_c]
            .reshape(S, DC, 128, H)
            .transpose(0, 2, 1, 3)
            .reshape(S, 128, DC * H)
        )
        in_maps.append(
            {
                "xAll": xT,
                "w1": np.ascontiguousarray(w1p),
                "b1": np.ascontiguousarray(b1[ts_c]),
                "w2": np.ascontiguousarray(w2b[ts_c]),
                "b2": np.ascontiguousarray(b2[ts_c]),
            }
        )
    return in_maps, (slot_tasks, idx, counts, M_slots)


def _unshard(results, meta, b_total=B):
    slot_tasks, idx, counts, M_slots = meta
    out = np.empty((b_total, C), dtype=np.float32)
    for c in range(N_CORES):
        yT = np.asarray(results[c]["outT"]).astype(np.float32)  # [C, M]
        off = 0
        for s in range(S):
            t = slot_tasks[s][c]
            cnt = counts[t]
            out[idx[s][c, :cnt]] = yT[:, off : off + cnt].T
            off += M_slots[s]
    return out


def kernel(x, task_id, W1, b1, W2, b2):
    in_maps, meta = _prepare(x, task_id, W1, b1, W2, b2)
    nc = _build(meta[3])
    try:
        res = run_bass_kernel_spmd(nc, in_maps, list(range(N_CORES)))
    except Exception:
        # transient NRT device hiccups (e.g. NRT_EXEC_UNIT_UNRECOVERABLE)
        # have been observed to succeed on retry
        res = run_bass_kernel_spmd(nc, in_maps, list(range(N_CORES)))
    return _unshard(res.results, meta, b_total=np.asarray(task_id).shape[0])


# revision 40
# speedup vs baseline: 1.0185x; 1.0185x over previous
"""MultiHeadClassifier (MoE routing) Trainium2 kernel.

Problem: B=65536 samples of dim D=1024, each routed by task_id to one of
T=16 two-layer heads (D->H=128 relu -> C=10). The dense reference computes
all 16 heads for every sample (275 GFLOP); here we route on the host and
compute only each sample's own head (~17 GFLOP), data-parallel with 2 task
slots per NeuronCore across 8 cores.

Strategy (v2, fp8-e3m4 moving operand):
  - Host: bucket samples by task. Tasks ranked by count; core c owns
    rank[c] (slot 0) and rank[15-c] (slot 1), so slot 0 is sized for the
    largest task and slot 1 for the 9th-largest: M0 + M1 < 2*max pad.
    Pad rows point at row 0 (results discarded on unshard).
  - x is quantized host-side to float8 e3m4 (4 mantissa bits; x~N(0,1)
    fits the +-15.5 range). Moving-operand DMA halves vs bf16; the PE
    upcasts e3m4 through the same e10m11 path as bf16 (1 col/cycle), so
    only the mantissa below bit 4 is lost. W1/W2 stay bf16 (their DMA is
    tiny); mixed-dtype matmul (bf16 stationary x e3m4 moving) is legal on
    TRN2. Measured end-to-end rel err ~1.4e-2 (vs 3.4e-3 all-bf16).
  - Host layout xAll [DC, 128, M]: chunk-major so each per-(block, dc)
    DMA reads 128 rows of XB contiguous bytes.
  - Device per (slot, m-tile of 512): 8 accumulating matmuls (W1 d-chunk
    [128,128] bf16 stationary, e3m4 x chunk [128,512] moving) -> PSUM;
    ScalarE fused bias+ReLU -> bf16 h; one matmul with W2 [128,10] ->
    PSUM [10,512]; DVE fused bias-add -> bf16 out tile; DMA out.
  - Host scatters per-task outputs back to the original order (f32).
"""

import sys

import numpy as np

for _p in ("/opt/trn_rl_repo", "/root/.axon_site/_ro/trn_rl_repo"):
    if _p not in sys.path:
        sys.path.append(_p)

import concourse.bacc as bacc
import concourse.mybir as mybir
from concourse.bass_utils import run_bass_kernel_spmd
from concourse.tile import TileContext

B, D, T, H, C = 65536, 1024, 16, 128, 10
N_CORES = 8
S = T // N_CORES  # task slots per core = 2
DC = D // 128  # d-chunks of 128 = 8
MT = 512  # m-tile (PSUM bank = 512 f32)
XB = 4096  # x DMA block (columns)
X_BUFS = 4
N_WARMUP = 8

_F32 = mybir.dt.float32
_BF16 = mybir.dt.bfloat16
_E3M4 = mybir.dt.float8e3

MM_DTYPE = "e3"  # kept for test.py compat


def _np_bf16():
    import ml_dtypes

    return np.dtype(ml_dtypes.bfloat16)


def _np_e3m4():
    import ml_dtypes

    return np.dtype(ml_dtypes.float8_e3m4)


def _chunks(total, step):
    out = []
    p = 0
    while p < total:
        c = min(step, total - p)
        out.append((p, c))
        p += c
    return out


def _blocks(total, step, ramp=False):
    """Like _chunks but ends with a small (<=512) final block so the
    compute tail after the last DMA is short. With ramp=True the first
    blocks are 512, 1024 so early compute is never starved waiting for
    a whole multi-MB block to stream in."""
    out = []
    p = 0
    rem = total
    if ramp:
        # halve the first block so the PE's first full-accumulation
        # subtile isn't gated on a whole 4MB block streaming in. Chunks
        # stay >=256KB: still transfer-bound, not issue-bound.
        for c in (XB // 2,):
            if rem <= c + 512:
                break
            out.append((p, c))
            p += c
            rem -= c
    while rem > 0:
        if rem <= 512 or rem <= step:
            c = rem
        elif rem <= step + 512:
            c = rem - 512
        else:
            c = step
        out.append((p, c))
        p += c
        rem -= c
    return out


def _blist(M_slots):
    """Global block list (slot, global col offset, len). Slot 1 is
    processed first (half-size blocks so the PE ramp tracks chunk
    delivery); the final block is slot 0's small (<=512) tail."""
    out = []
    offs = [0, M_slots[0]]
    for s in (1, 0):
        for x0, xl in _blocks(M_slots[s], XB // 2 if s == 1 else XB):
            out.append((s, offs[s] + x0, xl))
    return out


def _build(M_slots):
    """M_slots: per-slot padded column counts (M0, M1)."""
    M = sum(M_slots)
    nc = bacc.Bacc(None, target_bir_lowering=False)
    # x arrives packed per (block, chunk): each [128, xl] region is fully
    # contiguous in HBM (sequential addresses across partition rows), so
    # chunk DMAs read one clean linear span instead of 128 strided rows
    xAll = nc.declare_dram_parameter("xAll", [DC * 128 * M], _E3M4, isOutput=False)
    # w1 arrives host-repacked as [S, 128, DC*H]: partition-major, 2KB/row
    w1 = nc.declare_dram_parameter("w1", [S, 128, DC * H], _BF16, isOutput=False)
    b1 = nc.declare_dram_parameter("b1", [S, H], _F32, isOutput=False)
    w2 = nc.declare_dram_parameter("w2", [S, H, C], _BF16, isOutput=False)
    b2 = nc.declare_dram_parameter("b2", [S, C], _F32, isOutput=False)
    outT = nc.declare_dram_parameter("outT", [C, M], _BF16, isOutput=True)

    relu = mybir.ActivationFunctionType.Relu

    with TileContext(nc) as tc:
        with (
            tc.tile_pool(name="wpool", bufs=2) as wpool,
            tc.tile_pool(name="xpool", bufs=X_BUFS) as xpool,
            tc.tile_pool(name="hpool", bufs=8) as hpool,
            tc.tile_pool(name="opool", bufs=2) as opool,
            tc.tile_pool(name="warm", bufs=1) as warm,
            tc.tile_pool(name="psum1", bufs=5, space="PSUM") as psum1,
            tc.tile_pool(name="psum2", bufs=2, space="PSUM") as psum2,
            tc.tile_pool(name="psumw", bufs=1, space="PSUM") as psumw,
        ):  # PSUM banks: 5 + 2 + 1 = 8
            # PE warmup: dummy matmuls release the HAM clock-gate (~3.4us of
            # sustained PE busy) while the first x block streams in.
            wsrc = warm.tile([128, 256], _F32, tag="wsrc")
            nc.gpsimd.memset(wsrc[:], 0.0)
            wv_w = wsrc[:].bitcast(_BF16)  # [128, 512]
            wv_x = wsrc[:].bitcast(_E3M4)  # [128, 1024]
            wps = psumw.tile([128, MT], _F32, tag="wps")
            for _ in range(N_WARMUP):
                nc.tensor.matmul(
                    wps[:], wv_w[:, :128], wv_x[:, :MT], start=True, stop=True
                )
            # weight loads ride the scalar HWDGE ring. Slot 1 (processed
            # first) is hoisted to kernel start; slot 0's loads are emitted
            # after the first block's chunk DMAs (its compute starts ~15us
            # in) so they don't delay the first x chunks on this ring.
            wts = [None] * S

            def load_weights(s):
                w1t = wpool.tile([128, DC, H], _BF16, tag="w1", name=f"w1t{s}")
                nc.scalar.dma_start(
                    w1t, w1[s].rearrange("p (dc h) -> p dc h", dc=DC)
                )
                b1t = wpool.tile([H, 1], _F32, tag="b1", name=f"b1t{s}")
                nc.scalar.dma_start(b1t, b1[s][:, None])
                w2t = wpool.tile([H, C], _BF16, tag="w2", name=f"w2t{s}")
                nc.scalar.dma_start(w2t, w2[s])
                b2t = wpool.tile([C, 1], _F32, tag="b2", name=f"b2t{s}")
                nc.scalar.dma_start(b2t, b2[s][:, None])
                wts[s] = (w1t, b1t, w2t, b2t)

            load_weights(1)
            load_weights(0)

            blist = _blist(M_slots)

            # Hybrid schedule per block:
            #   - first <=4 subtiles run WAVE-style (dc-outer over 4 psum
            #     banks) so the PE consumes each x chunk ~860ns after it
            #     lands instead of idling until chunk 7 of the block is in
            #     (chunk delivery ~1.6us each, a full block ~12us);
            #   - remaining subtiles run subtile-major (dc-inner into one
            #     bank, drain to ACT immediately).
            # Layer-2 matmuls + bias-adds are deferred on a queue and
            # flushed in PAIRS after later L1 groups: the ACT latency hides
            # under the next 8 matmuls, and back-to-back MM2s share the W2
            # stationary so only the first pays the weight-switch bubble.
            pendq = []  # (ht, ot, m0, mt, w2t, b2t, out_dma or None)

            def flush_one():
                ht, ot_p, m0, mt, w2t_p, b2t_p, out_args = pendq.pop(0)
                ps2 = psum2.tile([C, MT], _F32, tag="ps2")
                nc.tensor.matmul(
                    ps2[:, :mt], w2t_p, ht[:, :mt], start=True, stop=True
                )
                nc.vector.tensor_tensor(
                    ot_p[:, m0 : m0 + mt],
                    ps2[:, :mt],
                    b2t_p.to_broadcast([C, mt]),
                    mybir.AluOpType.add,
                )
                if out_args is not None:
                    eng, o0, ol, otb, src0 = out_args
                    eng.dma_start(outT[:, o0 : o0 + ol], otb[:, src0 : src0 + ol])

            blk_base = 0
            for bi, (s, x0, xl) in enumerate(blist):
                if wts[s] is None:
                    load_weights(s)
                w1t, b1t, w2t, b2t = wts[s]
                # per-d-chunk tiles/DMAs: each a single contiguous HBM span,
                # chunk-granular deps, all on the SP HWDGE ring. (Splitting
                # across rings measured WORSE: scalar serializes against
                # ACTIVATEs, gpsimd's software descriptor generation is too
                # slow for 128-descriptor reads.)
                xts = []
                for dc in range(DC):
                    xtc = xpool.tile([128, XB], _E3M4, tag=f"x{dc}", name=f"x_{dc}")
                    off = blk_base + dc * 128 * xl
                    nc.sync.dma_start(
                        xtc[:, :xl],
                        xAll[off : off + 128 * xl].rearrange("(p m) -> p m", p=128),
                    )
                    xts.append(xtc)
                blk_base += DC * 128 * xl
                ot = opool.tile([C, XB], _BF16, tag="o")
                last_block = bi == len(blist) - 1
                # final block: 256-col subtiles + per-subtile out-DMA on the
                # (by then idle) sync HWDGE ring -> short kernel tail
                subs = _chunks(xl, 256 if last_block else MT)

                def mk_out(j, m0, mt):
                    if last_block:
                        return (nc.sync, x0 + m0, mt, ot, m0)
                    if j == len(subs) - 1:
                        # gpsimd (SWDGE): keeps the waiting out-DMA off the
                        # HWDGE rings so it can't block x-chunk DMAs
                        return (nc.gpsimd, x0, xl, ot, 0)
                    return None

                # drain ALL deferred L2 work before starting this block's
                # wave (every pending ACT is >=1 L1 group old, so no stall):
                # a short final block otherwise leaves the previous block's
                # out-DMA queued until the very end of the kernel
                while pendq:
                    flush_one()
                wave = subs[: min(4, len(subs))]
                rest = subs[len(wave) :]
                ps1s = [
                    psum1.tile([H, MT], _F32, tag="ps1", name=f"ps1_{j}")
                    for j in range(len(wave))
                ]
                for dc in range(DC):
                    for j, (m0, mt) in enumerate(wave):
                        nc.tensor.matmul(
                            ps1s[j][:, :mt],
                            w1t[:, dc, :],
                            xts[dc][:, m0 : m0 + mt],
                            start=(dc == 0),
                            stop=(dc == DC - 1),
                        )
                for j, (m0, mt) in enumerate(wave):
                    ht = hpool.tile([H, MT], _BF16, tag="h")
                    nc.scalar.activation(ht[:, :mt], ps1s[j][:, :mt], relu, bias=b1t)
                    pendq.append((ht, ot, m0, mt, w2t, b2t, mk_out(j, m0, mt)))
                for j0, (m0, mt) in enumerate(rest):
                    j = len(wave) + j0
                    ps1 = psum1.tile([H, MT], _F32, tag="ps1")
                    for dc in range(DC):
                        nc.tensor.matmul(
                            ps1[:, :mt],
                            w1t[:, dc, :],
                            xts[dc][:, m0 : m0 + mt],
                            start=(dc == 0),
                            stop=(dc == DC - 1),
                        )
                    for _ in range(min(2, len(pendq))):
                        flush_one()
                    ht = hpool.tile([H, MT], _BF16, tag="h")
                    nc.scalar.activation(ht[:, :mt], ps1[:, :mt], relu, bias=b1t)
                    pendq.append((ht, ot, m0, mt, w2t, b2t, mk_out(j, m0, mt)))
            while pendq:
                flush_one()
    nc.compile()
    return nc


def _prepare(x, task_id, W1, b1, W2, b2, mm_dtype=MM_DTYPE):
    """Host-side routing + quantization.

    Returns (in_maps, meta) where meta = (slot_tasks, idx, counts, M_slots).
    slot_tasks[s][c] = task owned by core c's slot s.
    """
    np_bf16 = _np_bf16()
    np_e3 = _np_e3m4()
    x = np.ascontiguousarray(np.asarray(x, dtype=np.float32))
    task_id = np.asarray(task_id).astype(np.int64)
    W1 = np.asarray(W1, dtype=np.float32)
    b1 = np.asarray(b1, dtype=np.float32)
    W2 = np.asarray(W2, dtype=np.float32)
    b2 = np.asarray(b2, dtype=np.float32)

    order = np.argsort(task_id, kind="stable")
    counts = np.bincount(task_id, minlength=T)
    starts = np.concatenate([[0], np.cumsum(counts)])

    # rank tasks by count desc; core c gets rank c (slot 0) and rank
    # 15-c (slot 1) so each slot's pad target is its own worst case
    ranks = np.argsort(-counts, kind="stable")
    slot_tasks = [
        [int(ranks[c]) for c in range(N_CORES)],
        [int(ranks[T - 1 - c]) for c in range(N_CORES)],
    ]
    c128 = lambda n: max(128, int(-(-int(n) // 128) * 128))
    M_slots = (
        c128(counts[ranks[0]]),
        c128(counts[ranks[N_CORES]]),
    )

    # idx[s][c] = sample rows for that slot's task, padded with row 0
    idx = [np.zeros((N_CORES, M_slots[s]), dtype=np.int64) for s in range(S)]
    for s in range(S):
        for c in range(N_CORES):
            t = slot_tasks[s][c]
            idx[s][c, : counts[t]] = order[starts[t] : starts[t + 1]]

    xq = x.astype(np_e3)  # RNE quantization; |x| << 15.5 so no overflow
    w1b = W1.astype(np_bf16)
    w2b = W2.astype(np_bf16)

    blist = _blist(M_slots)
    in_maps = []
    for c in range(N_CORES):
        ts_c = [slot_tasks[s][c] for s in range(S)]
        rows = np.concatenate([idx[s][c] for s in range(S)])  # [M]
        xg = xq[rows]  # [M, D] e3m4
        # chunk-major [DC, 128, M]: row (dc, p) holds x[:, dc*128+p];
        # then packed per (block, chunk) so each [128, xl] DMA region is
        # one contiguous HBM span
        xT = xg.reshape(-1, DC, 128).transpose(1, 2, 0)
        xT = np.concatenate(
            [xT[:, :, x0 : x0 + xl].reshape(-1) for (_s, x0, xl) in blist]
        )
        # repack W1 [D, H] -> [128, DC*H] (partition-major, 2KB DMA rows)
        w1p = (
            w1b[ts_c]
            .reshape(S, DC, 128, H)
            .transpose(0, 2, 1, 3)
            .reshape(S, 128, DC * H)
        )
        in_maps.append(
            {
                "xAll": xT,
                "w1": np.ascontiguousarray(w1p),
                "b1": np.ascontiguousarray(b1[ts_c]),
                "w2": np.ascontiguousarray(w2b[ts_c]),
                "b2": np.ascontiguousarray(b2[ts_c]),
            }
        )
    return in_maps, (slot_tasks, idx, counts, M_slots)


def _unshard(results, meta, b_total=B):
    slot_tasks, idx, counts, M_slots = meta
    out = np.empty((b_total, C), dtype=np.float32)
    for c in range(N_CORES):
        yT = np.asarray(results[c]["outT"]).astype(np.float32)  # [C, M]
        off = 0
        for s in range(S):
            t = slot_tasks[s][c]
            cnt = counts[t]
            out[idx[s][c, :cnt]] = yT[:, off : off + cnt].T
            off += M_slots[s]
    return out


def kernel(x, task_id, W1, b1, W2, b2):
    in_maps, meta = _prepare(x, task_id, W1, b1, W2, b2)
    nc = _build(meta[3])
    try:
        res = run_bass_kernel_spmd(nc, in_maps, list(range(N_CORES)))
    except Exception:
        # transient NRT device hiccups (e.g. NRT_EXEC_UNIT_UNRECOVERABLE)
        # have been observed to succeed on retry
        res = run_bass_kernel_spmd(nc, in_maps, list(range(N_CORES)))
    return _unshard(res.results, meta, b_total=np.asarray(task_id).shape[0])


# revision 42
# speedup vs baseline: 1.0204x; 1.0018x over previous
"""MultiHeadClassifier (MoE routing) Trainium2 kernel.

Problem: B=65536 samples of dim D=1024, each routed by task_id to one of
T=16 two-layer heads (D->H=128 relu -> C=10). The dense reference computes
all 16 heads for every sample (275 GFLOP); here we route on the host and
compute only each sample's own head (~17 GFLOP), data-parallel with 2 task
slots per NeuronCore across 8 cores.

Strategy (v2, fp8-e3m4 moving operand):
  - Host: bucket samples by task. Tasks ranked by count; core c owns
    rank[c] (slot 0) and rank[15-c] (slot 1), so slot 0 is sized for the
    largest task and slot 1 for the 9th-largest: M0 + M1 < 2*max pad.
    Pad rows point at row 0 (results discarded on unshard).
  - x is quantized host-side to float8 e3m4 (4 mantissa bits; x~N(0,1)
    fits the +-15.5 range). Moving-operand DMA halves vs bf16; the PE
    upcasts e3m4 through the same e10m11 path as bf16 (1 col/cycle), so
    only the mantissa below bit 4 is lost. W1/W2 stay bf16 (their DMA is
    tiny); mixed-dtype matmul (bf16 stationary x e3m4 moving) is legal on
    TRN2. Measured end-to-end rel err ~1.4e-2 (vs 3.4e-3 all-bf16).
  - Host layout xAll [DC, 128, M]: chunk-major so each per-(block, dc)
    DMA reads 128 rows of XB contiguous bytes.
  - Device per (slot, m-tile of 512): 8 accumulating matmuls (W1 d-chunk
    [128,128] bf16 stationary, e3m4 x chunk [128,512] moving) -> PSUM;
    ScalarE fused bias+ReLU -> bf16 h; one matmul with W2 [128,10] ->
    PSUM [10,512]; DVE fused bias-add -> bf16 out tile; DMA out.
  - Host scatters per-task outputs back to the original order (f32).
"""

import sys

import numpy as np

for _p in ("/opt/trn_rl_repo", "/root/.axon_site/_ro/trn_rl_repo"):
    if _p not in sys.path:
        sys.path.append(_p)

import concourse.bacc as bacc
import concourse.mybir as mybir
from concourse.bass_utils import run_bass_kernel_spmd
from concourse.tile import TileContext

B, D, T, H, C = 65536, 1024, 16, 128, 10
N_CORES = 8
S = T // N_CORES  # task slots per core = 2
DC = D // 128  # d-chunks of 128 = 8
MT = 512  # m-tile (PSUM bank = 512 f32)
XB = 4096  # x DMA block (columns)
X_BUFS = 4
N_WARMUP = 8

_F32 = mybir.dt.float32
_BF16 = mybir.dt.bfloat16
_E3M4 = mybir.dt.float8e3

MM_DTYPE = "e3"  # kept for test.py compat


def _np_bf16():
    import ml_dtypes

    return np.dtype(ml_dtypes.bfloat16)


def _np_e3m4():
    import ml_dtypes

    return np.dtype(ml_dtypes.float8_e3m4)


def _chunks(total, step):
    out = []
    p = 0
    while p < total:
        c = min(step, total - p)
        out.append((p, c))
        p += c
    return out


def _blocks(total, step, ramp=False):
    """Like _chunks but ends with a small (<=512) final block so the
    compute tail after the last DMA is short. With ramp=True the first
    blocks are 512, 1024 so early compute is never starved waiting for
    a whole multi-MB block to stream in."""
    out = []
    p = 0
    rem = total
    if ramp:
        # halve the first block so the PE's first full-accumulation
        # subtile isn't gated on a whole 4MB block streaming in. Chunks
        # stay >=256KB: still transfer-bound, not issue-bound.
        for c in (XB // 2,):
            if rem <= c + 512:
                break
            out.append((p, c))
            p += c
            rem -= c
    while rem > 0:
        if rem <= 512 or rem <= step:
            c = rem
        elif rem <= step + 512:
            c = rem - 512
        else:
            c = step
        out.append((p, c))
        p += c
        rem -= c
    return out


def _blist(M_slots):
    """Global block list (slot, global col offset, len). Slot 1 is
    processed first (half-size blocks so the PE ramp tracks chunk
    delivery); the final block is slot 0's small (<=512) tail."""
    out = []
    offs = [0, M_slots[0]]
    for s in (1, 0):
        for x0, xl in _blocks(M_slots[s], XB // 2 if s == 1 else XB):
            out.append((s, offs[s] + x0, xl))
    return out


def _build(M_slots):
    """M_slots: per-slot padded column counts (M0, M1)."""
    M = sum(M_slots)
    nc = bacc.Bacc(None, target_bir_lowering=False)
    # x arrives packed per (block, chunk): each [128, xl] region is fully
    # contiguous in HBM (sequential addresses across partition rows), so
    # chunk DMAs read one clean linear span instead of 128 strided rows
    xAll = nc.declare_dram_parameter("xAll", [DC * 128 * M], _E3M4, isOutput=False)
    # w1 arrives host-repacked as [S, 128, DC*H]: partition-major, 2KB/row
    w1 = nc.declare_dram_parameter("w1", [S, 128, DC * H], _BF16, isOutput=False)
    b1 = nc.declare_dram_parameter("b1", [S, H], _F32, isOutput=False)
    w2 = nc.declare_dram_parameter("w2", [S, H, C], _BF16, isOutput=False)
    b2 = nc.declare_dram_parameter("b2", [S, C], _F32, isOutput=False)
    outT = nc.declare_dram_parameter("outT", [C, M], _BF16, isOutput=True)

    relu = mybir.ActivationFunctionType.Relu

    with TileContext(nc) as tc:
        with (
            tc.tile_pool(name="wpool", bufs=2) as wpool,
            tc.tile_pool(name="xpool", bufs=X_BUFS) as xpool,
            tc.tile_pool(name="hpool", bufs=8) as hpool,
            tc.tile_pool(name="opool", bufs=2) as opool,
            tc.tile_pool(name="warm", bufs=1) as warm,
            tc.tile_pool(name="psum1", bufs=5, space="PSUM") as psum1,
            tc.tile_pool(name="psum2", bufs=2, space="PSUM") as psum2,
            tc.tile_pool(name="psumw", bufs=1, space="PSUM") as psumw,
        ):  # PSUM banks: 5 + 2 + 1 = 8
            # PE warmup: dummy matmuls release the HAM clock-gate (~3.4us of
            # sustained PE busy) while the first x block streams in.
            wsrc = warm.tile([128, 256], _F32, tag="wsrc")
            nc.gpsimd.memset(wsrc[:], 0.0)
            wv_w = wsrc[:].bitcast(_BF16)  # [128, 512]
            wv_x = wsrc[:].bitcast(_E3M4)  # [128, 1024]
            wps = psumw.tile([128, MT], _F32, tag="wps")
            for _ in range(N_WARMUP):
                nc.tensor.matmul(
                    wps[:], wv_w[:, :128], wv_x[:, :MT], start=True, stop=True
                )
            # weight loads ride the scalar HWDGE ring. Slot 1 (processed
            # first) is hoisted to kernel start; slot 0's loads are emitted
            # after the first block's chunk DMAs (its compute starts ~15us
            # in) so they don't delay the first x chunks on this ring.
            wts = [None] * S

            def load_weights(s):
                w1t = wpool.tile([128, DC, H], _BF16, tag="w1", name=f"w1t{s}")
                nc.scalar.dma_start(
                    w1t, w1[s].rearrange("p (dc h) -> p dc h", dc=DC)
                )
                b1t = wpool.tile([H, 1], _F32, tag="b1", name=f"b1t{s}")
                nc.scalar.dma_start(b1t, b1[s][:, None])
                w2t = wpool.tile([H, C], _BF16, tag="w2", name=f"w2t{s}")
                nc.scalar.dma_start(w2t, w2[s])
                b2t = wpool.tile([C, 1], _F32, tag="b2", name=f"b2t{s}")
                nc.scalar.dma_start(b2t, b2[s][:, None])
                wts[s] = (w1t, b1t, w2t, b2t)

            load_weights(1)
            load_weights(0)

            blist = _blist(M_slots)

            # Hybrid schedule per block:
            #   - first <=4 subtiles run WAVE-style (dc-outer over 4 psum
            #     banks) so the PE consumes each x chunk ~860ns after it
            #     lands instead of idling until chunk 7 of the block is in
            #     (chunk delivery ~1.6us each, a full block ~12us);
            #   - remaining subtiles run subtile-major (dc-inner into one
            #     bank, drain to ACT immediately).
            # Layer-2 matmuls + bias-adds are deferred on a queue and
            # flushed in PAIRS after later L1 groups: the ACT latency hides
            # under the next 8 matmuls, and back-to-back MM2s share the W2
            # stationary so only the first pays the weight-switch bubble.
            pendq = []  # (ht, ot, m0, mt, w2t, b2t, out_dma or None)

            def flush_one():
                ht, ot_p, m0, mt, w2t_p, b2t_p, out_args = pendq.pop(0)
                ps2 = psum2.tile([C, MT], _F32, tag="ps2")
                nc.tensor.matmul(
                    ps2[:, :mt], w2t_p, ht[:, :mt], start=True, stop=True
                )
                nc.vector.tensor_tensor(
                    ot_p[:, m0 : m0 + mt],
                    ps2[:, :mt],
                    b2t_p.to_broadcast([C, mt]),
                    mybir.AluOpType.add,
                )
                if out_args is not None:
                    eng, o0, ol, otb, src0 = out_args
                    eng.dma_start(outT[:, o0 : o0 + ol], otb[:, src0 : src0 + ol])

            blk_base = 0
            for bi, (s, x0, xl) in enumerate(blist):
                if wts[s] is None:
                    load_weights(s)
                w1t, b1t, w2t, b2t = wts[s]
                # per-d-chunk tiles/DMAs: each a single contiguous HBM span,
                # chunk-granular deps, all on the SP HWDGE ring. (Splitting
                # across rings measured WORSE: scalar serializes against
                # ACTIVATEs, gpsimd's software descriptor generation is too
                # slow for 128-descriptor reads.)
                xts = []
                for dc in range(DC):
                    xtc = xpool.tile([128, XB], _E3M4, tag=f"x{dc}", name=f"x_{dc}")
                    off = blk_base + dc * 128 * xl
                    nc.sync.dma_start(
                        xtc[:, :xl],
                        xAll[off : off + 128 * xl].rearrange("(p m) -> p m", p=128),
                    )
                    xts.append(xtc)
                blk_base += DC * 128 * xl
                ot = opool.tile([C, XB], _BF16, tag="o")
                last_block = bi == len(blist) - 1
                # final block: 256-col subtiles + per-subtile out-DMA on the
                # (by then idle) sync HWDGE ring -> short kernel tail
                subs = _chunks(xl, 256 if last_block else MT)

                def mk_out(j, m0, mt):
                    if last_block:
                        # alternate rings so the final per-subtile out-DMAs
                        # overlap instead of queueing on one ring
                        eng = nc.sync if j % 2 else nc.scalar
                        return (eng, x0 + m0, mt, ot, m0)
                    if j == len(subs) - 1:
                        # gpsimd (SWDGE): keeps the waiting out-DMA off the
                        # HWDGE rings so it can't block x-chunk DMAs
                        return (nc.gpsimd, x0, xl, ot, 0)
                    return None

                # drain ALL deferred L2 work before starting this block's
                # wave (every pending ACT is >=1 L1 group old, so no stall):
                # a short final block otherwise leaves the previous block's
                # out-DMA queued until the very end of the kernel
                while pendq:
                    flush_one()
                # final block runs subtile-major (wave=1): the first tail
                # subtile's L2+out-DMA then overlaps the second's compute
                wave = subs[:1] if last_block else subs[: min(4, len(subs))]
                rest = subs[len(wave) :]
                ps1s = [
                    psum1.tile([H, MT], _F32, tag="ps1", name=f"ps1_{j}")
                    for j in range(len(wave))
                ]
                for dc in range(DC):
                    for j, (m0, mt) in enumerate(wave):
                        nc.tensor.matmul(
                            ps1s[j][:, :mt],
                            w1t[:, dc, :],
                            xts[dc][:, m0 : m0 + mt],
                            start=(dc == 0),
                            stop=(dc == DC - 1),
                        )
                for j, (m0, mt) in enumerate(wave):
                    ht = hpool.tile([H, MT], _BF16, tag="h")
                    nc.scalar.activation(ht[:, :mt], ps1s[j][:, :mt], relu, bias=b1t)
                    pendq.append((ht, ot, m0, mt, w2t, b2t, mk_out(j, m0, mt)))
                for j0, (m0, mt) in enumerate(rest):
                    j = len(wave) + j0
                    ps1 = psum1.tile([H, MT], _F32, tag="ps1")
                    for dc in range(DC):
                        nc.tensor.matmul(
                            ps1[:, :mt],
                            w1t[:, dc, :],
                            xts[dc][:, m0 : m0 + mt],
                            start=(dc == 0),
                            stop=(dc == DC - 1),
                        )
                    for _ in range(min(2, len(pendq))):
                        flush_one()
                    ht = hpool.tile([H, MT], _BF16, tag="h")
                    nc.scalar.activation(ht[:, :mt], ps1[:, :mt], relu, bias=b1t)
                    pendq.append((ht, ot, m0, mt, w2t, b2t, mk_out(j, m0, mt)))
            while pendq:
                flush_one()
    nc.compile()
    return nc


def _prepare(x, task_id, W1, b1, W2, b2, mm_dtype=MM_DTYPE):
    """Host-side routing + quantization.

    Returns (in_maps, meta) where meta = (slot_tasks, idx, counts, M_slots).
    slot_tasks[s][c] = task owned by core c's slot s.
    """
    np_bf16 = _np_bf16()
    np_e3 = _np_e3m4()
    x = np.ascontiguousarray(np.asarray(x, dtype=np.float32))
    task_id = np.asarray(task_id).astype(np.int64)
    W1 = np.asarray(W1, dtype=np.float32)
    b1 = np.asarray(b1, dtype=np.float32)
    W2 = np.asarray(W2, dtype=np.float32)
    b2 = np.asarray(b2, dtype=np.float32)

    order = np.argsort(task_id, kind="stable")
    counts = np.bincount(task_id, minlength=T)
    starts = np.concatenate([[0], np.cumsum(counts)])

    # rank tasks by count desc; core c gets rank c (slot 0) and rank
    # 15-c (slot 1) so each slot's pad target is its own worst case
    ranks = np.argsort(-counts, kind="stable")
    slot_tasks = [
        [int(ranks[c]) for c in range(N_CORES)],
        [int(ranks[T - 1 - c]) for c in range(N_CORES)],
    ]
    c128 = lambda n: max(128, int(-(-int(n) // 128) * 128))
    M_slots = (
        c128(counts[ranks[0]]),
        c128(counts[ranks[N_CORES]]),
    )

    # idx[s][c] = sample rows for that slot's task, padded with row 0
    idx = [np.zeros((N_CORES, M_slots[s]), dtype=np.int64) for s in range(S)]
    for s in range(S):
        for c in range(N_CORES):
            t = slot_tasks[s][c]
            idx[s][c, : counts[t]] = order[starts[t] : starts[t + 1]]

    xq = x.astype(np_e3)  # RNE quantization; |x| << 15.5 so no overflow
    w1b = W1.astype(np_bf16)
    w2b = W2.astype(np_bf16)

    blist = _blist(M_slots)
    in_maps = []
    for c in range(N_CORES):
        ts_c = [slot_tasks[s][c] for s in range(S)]
        rows = np.concatenate([idx[s][c] for s in range(S)])  # [M]
        xg = xq[rows]  # [M, D] e3m4
        # chunk-major [DC, 128, M]: row (dc, p) holds x[:, dc*128+p];
        # then packed per (block, chunk) so each [128, xl] DMA region is
        # one contiguous HBM span
        xT = xg.reshape(-1, DC, 128).transpose(1, 2, 0)
        xT = np.concatenate(
            [xT[:, :, x0 : x0 + xl].reshape(-1) for (_s, x0, xl) in blist]
        )
        # repack W1 [D, H] -> [128, DC*H] (partition-major, 2KB DMA rows)
        w1p = (
            w1b[ts_c]
            .reshape(S, DC, 128, H)
            .transpose(0, 2, 1, 3)
            .reshape(S, 128, DC * H)
        )
        in_maps.append(
            {
                "xAll": xT,
                "w1": np.ascontiguousarray(w1p),
                "b1": np.ascontiguousarray(b1[ts_c]),
                "w2": np.ascontiguousarray(w2b[ts_c]),
                "b2": np.ascontiguousarray(b2[ts_c]),
            }
        )
    return in_maps, (slot_tasks, idx, counts, M_slots)


def _unshard(results, meta, b_total=B):
    slot_tasks, idx, counts, M_slots = meta
    out = np.empty((b_total, C), dtype=np.float32)
    for c in range(N_CORES):
        yT = np.asarray(results[c]["outT"]).astype(np.float32)  # [C, M]
        off = 0
        for s in range(S):
            t = slot_tasks[s][c]
            cnt = counts[t]
            out[idx[s][c, :cnt]] = yT[:, off : off + cnt].T
            off += M_slots[s]
    return out


def kernel(x, task_id, W1, b1, W2, b2):
    in_maps, meta = _prepare(x, task_id, W1, b1, W2, b2)
    nc = _build(meta[3])
    try:
        res = run_bass_kernel_spmd(nc, in_maps, list(range(N_CORES)))
    except Exception:
        # transient NRT device hiccups (e.g. NRT_EXEC_UNIT_UNRECOVERABLE)
        # have been observed to succeed on retry
        res = run_bass_kernel_spmd(nc, in_maps, list(range(N_CORES)))
    return _unshard(res.results, meta, b_total=np.asarray(task_id).shape[0])


# revision 43
# speedup vs baseline: 1.0339x; 1.0133x over previous
"""MultiHeadClassifier (MoE routing) Trainium2 kernel.

Problem: B=65536 samples of dim D=1024, each routed by task_id to one of
T=16 two-layer heads (D->H=128 relu -> C=10). The dense reference computes
all 16 heads for every sample (275 GFLOP); here we route on the host and
compute only each sample's own head (~17 GFLOP), data-parallel with 2 task
slots per NeuronCore across 8 cores.

Strategy (v2, fp8-e3m4 moving operand):
  - Host: bucket samples by task. Tasks ranked by count; core c owns
    rank[c] (slot 0) and rank[15-c] (slot 1), so slot 0 is sized for the
    largest task and slot 1 for the 9th-largest: M0 + M1 < 2*max pad.
    Pad rows point at row 0 (results discarded on unshard).
  - x is quantized host-side to float8 e3m4 (4 mantissa bits; x~N(0,1)
    fits the +-15.5 range). Moving-operand DMA halves vs bf16; the PE
    upcasts e3m4 through the same e10m11 path as bf16 (1 col/cycle), so
    only the mantissa below bit 4 is lost. W1/W2 stay bf16 (their DMA is
    tiny); mixed-dtype matmul (bf16 stationary x e3m4 moving) is legal on
    TRN2. Measured end-to-end rel err ~1.4e-2 (vs 3.4e-3 all-bf16).
  - Host layout xAll [DC, 128, M]: chunk-major so each per-(block, dc)
    DMA reads 128 rows of XB contiguous bytes.
  - Device per (slot, m-tile of 512): 8 accumulating matmuls (W1 d-chunk
    [128,128] bf16 stationary, e3m4 x chunk [128,512] moving) -> PSUM;
    ScalarE fused bias+ReLU -> bf16 h; one matmul with W2 [128,10] ->
    PSUM [10,512]; DVE fused bias-add -> bf16 out tile; DMA out.
  - Host scatters per-task outputs back to the original order (f32).
"""

import sys

import numpy as np

for _p in ("/opt/trn_rl_repo", "/root/.axon_site/_ro/trn_rl_repo"):
    if _p not in sys.path:
        sys.path.append(_p)

import concourse.bacc as bacc
import concourse.mybir as mybir
from concourse.bass_utils import run_bass_kernel_spmd
from concourse.tile import TileContext

B, D, T, H, C = 65536, 1024, 16, 128, 10
N_CORES = 8
S = T // N_CORES  # task slots per core = 2
DC = D // 128  # d-chunks of 128 = 8
MT = 512  # m-tile (PSUM bank = 512 f32)
XB = 4096  # x DMA block (columns)
X_BUFS = 4
N_WARMUP = 8

_F32 = mybir.dt.float32
_BF16 = mybir.dt.bfloat16
_E3M4 = mybir.dt.float8e3

MM_DTYPE = "e3"  # kept for test.py compat


def _np_bf16():
    import ml_dtypes

    return np.dtype(ml_dtypes.bfloat16)


def _np_e3m4():
    import ml_dtypes

    return np.dtype(ml_dtypes.float8_e3m4)


def _chunks(total, step):
    out = []
    p = 0
    while p < total:
        c = min(step, total - p)
        out.append((p, c))
        p += c
    return out


def _blocks(total, step, ramp=False):
    """Like _chunks but ends with a small (<=512) final block so the
    compute tail after the last DMA is short. With ramp=True the first
    blocks are 512, 1024 so early compute is never starved waiting for
    a whole multi-MB block to stream in."""
    out = []
    p = 0
    rem = total
    if ramp:
        # halve the first block so the PE's first full-accumulation
        # subtile isn't gated on a whole 4MB block streaming in. Chunks
        # stay >=256KB: still transfer-bound, not issue-bound.
        for c in (XB // 2,):
            if rem <= c + 512:
                break
            out.append((p, c))
            p += c
            rem -= c
    while rem > 0:
        if rem <= 512 or rem <= step:
            c = rem
        elif rem <= step + 512:
            c = rem - 512
        else:
            c = step
        out.append((p, c))
        p += c
        rem -= c
    return out


def _blist(M_slots):
    """Global block list (slot, global col offset, len). Slot 1 is
    processed first (half-size blocks so the PE ramp tracks chunk
    delivery); the final block is slot 0's small (<=512) tail."""
    out = []
    offs = [0, M_slots[0]]
    for s in (1, 0):
        for x0, xl in _blocks(M_slots[s], XB // 2):
            out.append((s, offs[s] + x0, xl))
    return out


def _build(M_slots):
    """M_slots: per-slot padded column counts (M0, M1)."""
    M = sum(M_slots)
    nc = bacc.Bacc(None, target_bir_lowering=False)
    # x arrives packed per (block, chunk): each [128, xl] region is fully
    # contiguous in HBM (sequential addresses across partition rows), so
    # chunk DMAs read one clean linear span instead of 128 strided rows
    xAll = nc.declare_dram_parameter("xAll", [DC * 128 * M], _E3M4, isOutput=False)
    # w1 arrives host-repacked as [S, 128, DC*H]: partition-major, 2KB/row
    w1 = nc.declare_dram_parameter("w1", [S, 128, DC * H], _BF16, isOutput=False)
    b1 = nc.declare_dram_parameter("b1", [S, H], _F32, isOutput=False)
    w2 = nc.declare_dram_parameter("w2", [S, H, C], _BF16, isOutput=False)
    b2 = nc.declare_dram_parameter("b2", [S, C], _F32, isOutput=False)
    outT = nc.declare_dram_parameter("outT", [C, M], _BF16, isOutput=True)

    relu = mybir.ActivationFunctionType.Relu

    with TileContext(nc) as tc:
        with (
            tc.tile_pool(name="wpool", bufs=2) as wpool,
            tc.tile_pool(name="xpool", bufs=X_BUFS) as xpool,
            tc.tile_pool(name="hpool", bufs=8) as hpool,
            tc.tile_pool(name="opool", bufs=2) as opool,
            tc.tile_pool(name="warm", bufs=1) as warm,
            tc.tile_pool(name="psum1", bufs=5, space="PSUM") as psum1,
            tc.tile_pool(name="psum2", bufs=2, space="PSUM") as psum2,
            tc.tile_pool(name="psumw", bufs=1, space="PSUM") as psumw,
        ):  # PSUM banks: 5 + 2 + 1 = 8
            # PE warmup: dummy matmuls release the HAM clock-gate (~3.4us of
            # sustained PE busy) while the first x block streams in.
            wsrc = warm.tile([128, 256], _F32, tag="wsrc")
            nc.gpsimd.memset(wsrc[:], 0.0)
            wv_w = wsrc[:].bitcast(_BF16)  # [128, 512]
            wv_x = wsrc[:].bitcast(_E3M4)  # [128, 1024]
            wps = psumw.tile([128, MT], _F32, tag="wps")
            for _ in range(N_WARMUP):
                nc.tensor.matmul(
                    wps[:], wv_w[:, :128], wv_x[:, :MT], start=True, stop=True
                )
            # weight loads ride the scalar HWDGE ring. Slot 1 (processed
            # first) is hoisted to kernel start; slot 0's loads are emitted
            # after the first block's chunk DMAs (its compute starts ~15us
            # in) so they don't delay the first x chunks on this ring.
            wts = [None] * S

            def load_weights(s):
                w1t = wpool.tile([128, DC, H], _BF16, tag="w1", name=f"w1t{s}")
                nc.scalar.dma_start(
                    w1t, w1[s].rearrange("p (dc h) -> p dc h", dc=DC)
                )
                b1t = wpool.tile([H, 1], _F32, tag="b1", name=f"b1t{s}")
                nc.scalar.dma_start(b1t, b1[s][:, None])
                w2t = wpool.tile([H, C], _BF16, tag="w2", name=f"w2t{s}")
                nc.scalar.dma_start(w2t, w2[s])
                b2t = wpool.tile([C, 1], _F32, tag="b2", name=f"b2t{s}")
                nc.scalar.dma_start(b2t, b2[s][:, None])
                wts[s] = (w1t, b1t, w2t, b2t)

            load_weights(1)
            load_weights(0)

            blist = _blist(M_slots)

            # Hybrid schedule per block:
            #   - first <=4 subtiles run WAVE-style (dc-outer over 4 psum
            #     banks) so the PE consumes each x chunk ~860ns after it
            #     lands instead of idling until chunk 7 of the block is in
            #     (chunk delivery ~1.6us each, a full block ~12us);
            #   - remaining subtiles run subtile-major (dc-inner into one
            #     bank, drain to ACT immediately).
            # Layer-2 matmuls + bias-adds are deferred on a queue and
            # flushed in PAIRS after later L1 groups: the ACT latency hides
            # under the next 8 matmuls, and back-to-back MM2s share the W2
            # stationary so only the first pays the weight-switch bubble.
            pendq = []  # (ht, ot, m0, mt, w2t, b2t, out_dma or None)

            def flush_one():
                ht, ot_p, m0, mt, w2t_p, b2t_p, out_args = pendq.pop(0)
                ps2 = psum2.tile([C, MT], _F32, tag="ps2")
                nc.tensor.matmul(
                    ps2[:, :mt], w2t_p, ht[:, :mt], start=True, stop=True
                )
                nc.vector.tensor_tensor(
                    ot_p[:, m0 : m0 + mt],
                    ps2[:, :mt],
                    b2t_p.to_broadcast([C, mt]),
                    mybir.AluOpType.add,
                )
                if out_args is not None:
                    eng, o0, ol, otb, src0 = out_args
                    eng.dma_start(outT[:, o0 : o0 + ol], otb[:, src0 : src0 + ol])

            blk_base = 0
            for bi, (s, x0, xl) in enumerate(blist):
                if wts[s] is None:
                    load_weights(s)
                w1t, b1t, w2t, b2t = wts[s]
                # per-d-chunk tiles/DMAs: each a single contiguous HBM span,
                # chunk-granular deps, all on the SP HWDGE ring. (Splitting
                # across rings measured WORSE: scalar serializes against
                # ACTIVATEs, gpsimd's software descriptor generation is too
                # slow for 128-descriptor reads.)
                xts = []
                for dc in range(DC):
                    xtc = xpool.tile([128, XB], _E3M4, tag=f"x{dc}", name=f"x_{dc}")
                    off = blk_base + dc * 128 * xl
                    nc.sync.dma_start(
                        xtc[:, :xl],
                        xAll[off : off + 128 * xl].rearrange("(p m) -> p m", p=128),
                    )
                    xts.append(xtc)
                blk_base += DC * 128 * xl
                ot = opool.tile([C, XB], _BF16, tag="o")
                last_block = bi == len(blist) - 1
                # final block: 256-col subtiles + per-subtile out-DMA on the
                # (by then idle) sync HWDGE ring -> short kernel tail
                subs = _chunks(xl, 256 if last_block else MT)

                def mk_out(j, m0, mt):
                    if last_block:
                        # alternate rings so the final per-subtile out-DMAs
                        # overlap instead of queueing on one ring
                        eng = nc.sync if j % 2 else nc.scalar
                        return (eng, x0 + m0, mt, ot, m0)
                    if j == len(subs) - 1:
                        # gpsimd (SWDGE): keeps the waiting out-DMA off the
                        # HWDGE rings so it can't block x-chunk DMAs
                        return (nc.gpsimd, x0, xl, ot, 0)
                    return None

                # drain ALL deferred L2 work before starting this block's
                # wave (every pending ACT is >=1 L1 group old, so no stall):
                # a short final block otherwise leaves the previous block's
                # out-DMA queued until the very end of the kernel
                while pendq:
                    flush_one()
                # final block runs subtile-major (wave=1): the first tail
                # subtile's L2+out-DMA then overlaps the second's compute
                wave = subs[:1] if last_block else subs[: min(4, len(subs))]
                rest = subs[len(wave) :]
                ps1s = [
                    psum1.tile([H, MT], _F32, tag="ps1", name=f"ps1_{j}")
                    for j in range(len(wave))
                ]
                for dc in range(DC):
                    for j, (m0, mt) in enumerate(wave):
                        nc.tensor.matmul(
                            ps1s[j][:, :mt],
                            w1t[:, dc, :],
                            xts[dc][:, m0 : m0 + mt],
                            start=(dc == 0),
                            stop=(dc == DC - 1),
                        )
                for j, (m0, mt) in enumerate(wave):
                    ht = hpool.tile([H, MT], _BF16, tag="h")
                    nc.scalar.activation(ht[:, :mt], ps1s[j][:, :mt], relu, bias=b1t)
                    pendq.append((ht, ot, m0, mt, w2t, b2t, mk_out(j, m0, mt)))
                for j0, (m0, mt) in enumerate(rest):
                    j = len(wave) + j0
                    ps1 = psum1.tile([H, MT], _F32, tag="ps1")
                    for dc in range(DC):
                        nc.tensor.matmul(
                            ps1[:, :mt],
                            w1t[:, dc, :],
                            xts[dc][:, m0 : m0 + mt],
                            start=(dc == 0),
                            stop=(dc == DC - 1),
                        )
                    for _ in range(min(2, len(pendq))):
                        flush_one()
                    ht = hpool.tile([H, MT], _BF16, tag="h")
                    nc.scalar.activation(ht[:, :mt], ps1[:, :mt], relu, bias=b1t)
                    pendq.append((ht, ot, m0, mt, w2t, b2t, mk_out(j, m0, mt)))
            while pendq:
                flush_one()
    nc.compile()
    return nc


def _prepare(x, task_id, W1, b1, W2, b2, mm_dtype=MM_DTYPE):
    """Host-side routing + quantization.

    Returns (in_maps, meta) where meta = (slot_tasks, idx, counts, M_slots).
    slot_tasks[s][c] = task owned by core c's slot s.
    """
    np_bf16 = _np_bf16()
    np_e3 = _np_e3m4()
    x = np.ascontiguousarray(np.asarray(x, dtype=np.float32))
    task_id = np.asarray(task_id).astype(np.int64)
    W1 = np.asarray(W1, dtype=np.float32)
    b1 = np.asarray(b1, dtype=np.float32)
    W2 = np.asarray(W2, dtype=np.float32)
    b2 = np.asarray(b2, dtype=np.float32)

    order = np.argsort(task_id, kind="stable")
    counts = np.bincount(task_id, minlength=T)
    starts = np.concatenate([[0], np.cumsum(counts)])

    # rank tasks by count desc; core c gets rank c (slot 0) and rank
    # 15-c (slot 1) so each slot's pad target is its own worst case
    ranks = np.argsort(-counts, kind="stable")
    slot_tasks = [
        [int(ranks[c]) for c in range(N_CORES)],
        [int(ranks[T - 1 - c]) for c in range(N_CORES)],
    ]
    c128 = lambda n: max(128, int(-(-int(n) // 128) * 128))
    M_slots = (
        c128(counts[ranks[0]]),
        c128(counts[ranks[N_CORES]]),
    )

    # idx[s][c] = sample rows for that slot's task, padded with row 0
    idx = [np.zeros((N_CORES, M_slots[s]), dtype=np.int64) for s in range(S)]
    for s in range(S):
        for c in range(N_CORES):
            t = slot_tasks[s][c]
            idx[s][c, : counts[t]] = order[starts[t] : starts[t + 1]]

    xq = x.astype(np_e3)  # RNE quantization; |x| << 15.5 so no overflow
    w1b = W1.astype(np_bf16)
    w2b = W2.astype(np_bf16)

    blist = _blist(M_slots)
    in_maps = []
    for c in range(N_CORES):
        ts_c = [slot_tasks[s][c] for s in range(S)]
        rows = np.concatenate([idx[s][c] for s in range(S)])  # [M]
        xg = xq[rows]  # [M, D] e3m4
        # chunk-major [DC, 128, M]: row (dc, p) holds x[:, dc*128+p];
        # then packed per (block, chunk) so each [128, xl] DMA region is
        # one contiguous HBM span
        xT = xg.reshape(-1, DC, 128).transpose(1, 2, 0)
        xT = np.concatenate(
            [xT[:, :, x0 : x0 + xl].reshape(-1) for (_s, x0, xl) in blist]
        )
        # repack W1 [D, H] -> [128, DC*H] (partition-major, 2KB DMA rows)
        w1p = (
            w1b[ts_c]
            .reshape(S, DC, 128, H)
            .transpose(0, 2, 1, 3)
            .reshape(S, 128, DC * H)
        )
        in_maps.append(
            {
                "xAll": xT,
                "w1": np.ascontiguousarray(w1p),
                "b1": np.ascontiguousarray(b1[ts_c]),
                "w2": np.ascontiguousarray(w2b[ts_c]),
                "b2": np.ascontiguousarray(b2[ts_c]),
            }
        )
    return in_maps, (slot_tasks, idx, counts, M_slots)


def _unshard(results, meta, b_total=B):
    slot_tasks, idx, counts, M_slots = meta
    out = np.empty((b_total, C), dtype=np.float32)
    for c in range(N_CORES):
        yT = np.asarray(results[c]["outT"]).astype(np.float32)  # [C, M]
        off = 0
        for s in range(S):
            t = slot_tasks[s][c]
            cnt = counts[t]
            out[idx[s][c, :cnt]] = yT[:, off : off + cnt].T
            off += M_slots[s]
    return out


def kernel(x, task_id, W1, b1, W2, b2):
    in_maps, meta = _prepare(x, task_id, W1, b1, W2, b2)
    nc = _build(meta[3])
    try:
        res = run_bass_kernel_spmd(nc, in_maps, list(range(N_CORES)))
    except Exception:
        # transient NRT device hiccups (e.g. NRT_EXEC_UNIT_UNRECOVERABLE)
        # have been observed to succeed on retry
        res = run_bass_kernel_spmd(nc, in_maps, list(range(N_CORES)))
    return _unshard(res.results, meta, b_total=np.asarray(task_id).shape[0])


# revision 44
# speedup vs baseline: 1.0406x; 1.0064x over previous
"""MultiHeadClassifier (MoE routing) Trainium2 kernel.

Problem: B=65536 samples of dim D=1024, each routed by task_id to one of
T=16 two-layer heads (D->H=128 relu -> C=10). The dense reference computes
all 16 heads for every sample (275 GFLOP); here we route on the host and
compute only each sample's own head (~17 GFLOP), data-parallel with 2 task
slots per NeuronCore across 8 cores.

Strategy (v2, fp8-e3m4 moving operand):
  - Host: bucket samples by task. Tasks ranked by count; core c owns
    rank[c] (slot 0) and rank[15-c] (slot 1), so slot 0 is sized for the
    largest task and slot 1 for the 9th-largest: M0 + M1 < 2*max pad.
    Pad rows point at row 0 (results discarded on unshard).
  - x is quantized host-side to float8 e3m4 (4 mantissa bits; x~N(0,1)
    fits the +-15.5 range). Moving-operand DMA halves vs bf16; the PE
    upcasts e3m4 through the same e10m11 path as bf16 (1 col/cycle), so
    only the mantissa below bit 4 is lost. W1/W2 stay bf16 (their DMA is
    tiny); mixed-dtype matmul (bf16 stationary x e3m4 moving) is legal on
    TRN2. Measured end-to-end rel err ~1.4e-2 (vs 3.4e-3 all-bf16).
  - Host layout xAll [DC, 128, M]: chunk-major so each per-(block, dc)
    DMA reads 128 rows of XB contiguous bytes.
  - Device per (slot, m-tile of 512): 8 accumulating matmuls (W1 d-chunk
    [128,128] bf16 stationary, e3m4 x chunk [128,512] moving) -> PSUM;
    ScalarE fused bias+ReLU -> bf16 h; one matmul with W2 [128,10] ->
    PSUM [10,512]; DVE fused bias-add -> bf16 out tile; DMA out.
  - Host scatters per-task outputs back to the original order (f32).
"""

import sys

import numpy as np

for _p in ("/opt/trn_rl_repo", "/root/.axon_site/_ro/trn_rl_repo"):
    if _p not in sys.path:
        sys.path.append(_p)

import concourse.bacc as bacc
import concourse.mybir as mybir
from concourse.bass_utils import run_bass_kernel_spmd
from concourse.tile import TileContext

B, D, T, H, C = 65536, 1024, 16, 128, 10
N_CORES = 8
S = T // N_CORES  # task slots per core = 2
DC = D // 128  # d-chunks of 128 = 8
MT = 512  # m-tile (PSUM bank = 512 f32)
XB = 4096  # x DMA block (columns)
X_BUFS = 4
N_WARMUP = 8

_F32 = mybir.dt.float32
_BF16 = mybir.dt.bfloat16
_E3M4 = mybir.dt.float8e3

MM_DTYPE = "e3"  # kept for test.py compat


def _np_bf16():
    import ml_dtypes

    return np.dtype(ml_dtypes.bfloat16)


def _np_e3m4():
    import ml_dtypes

    return np.dtype(ml_dtypes.float8_e3m4)


def _chunks(total, step):
    out = []
    p = 0
    while p < total:
        c = min(step, total - p)
        out.append((p, c))
        p += c
    return out


def _blocks(total, step, ramp=False):
    """Like _chunks but ends with a small (<=512) final block so the
    compute tail after the last DMA is short. With ramp=True the first
    blocks are 512, 1024 so early compute is never starved waiting for
    a whole multi-MB block to stream in."""
    out = []
    p = 0
    rem = total
    if ramp:
        # halve the first block so the PE's first full-accumulation
        # subtile isn't gated on a whole 4MB block streaming in. Chunks
        # stay >=256KB: still transfer-bound, not issue-bound.
        for c in (XB // 2,):
            if rem <= c + 512:
                break
            out.append((p, c))
            p += c
            rem -= c
    while rem > 0:
        if rem <= 512 or rem <= step:
            c = rem
        elif rem <= step + 512:
            c = rem - 512
        else:
            c = step
        out.append((p, c))
        p += c
        rem -= c
    return out


def _blist(M_slots):
    """Global block list (slot, global col offset, len). Slot 1 is
    processed first (half-size blocks so the PE ramp tracks chunk
    delivery); the final block is slot 0's small (<=512) tail."""
    out = []
    offs = [0, M_slots[0]]
    for s in (1, 0):
        for x0, xl in _blocks(M_slots[s], XB // 2):
            out.append((s, offs[s] + x0, xl))
    return out


def _build(M_slots):
    """M_slots: per-slot padded column counts (M0, M1)."""
    M = sum(M_slots)
    nc = bacc.Bacc(None, target_bir_lowering=False)
    # x arrives packed per (block, chunk): each [128, xl] region is fully
    # contiguous in HBM (sequential addresses across partition rows), so
    # chunk DMAs read one clean linear span instead of 128 strided rows
    xAll = nc.declare_dram_parameter("xAll", [DC * 128 * M], _E3M4, isOutput=False)
    # w1 arrives host-repacked as [S, 128, DC*H]: partition-major, 2KB/row
    w1 = nc.declare_dram_parameter("w1", [S, 128, DC * H], _BF16, isOutput=False)
    b1 = nc.declare_dram_parameter("b1", [S, H], _F32, isOutput=False)
    w2 = nc.declare_dram_parameter("w2", [S, H, C], _BF16, isOutput=False)
    b2 = nc.declare_dram_parameter("b2", [S, C], _F32, isOutput=False)
    outT = nc.declare_dram_parameter("outT", [C, M], _BF16, isOutput=True)

    relu = mybir.ActivationFunctionType.Relu

    with TileContext(nc) as tc:
        with (
            tc.tile_pool(name="wpool", bufs=2) as wpool,
            tc.tile_pool(name="xpool", bufs=X_BUFS) as xpool,
            tc.tile_pool(name="hpool", bufs=8) as hpool,
            tc.tile_pool(name="opool", bufs=2) as opool,
            tc.tile_pool(name="warm", bufs=1) as warm,
            tc.tile_pool(name="psum1", bufs=5, space="PSUM") as psum1,
            tc.tile_pool(name="psum2", bufs=2, space="PSUM") as psum2,
            tc.tile_pool(name="psumw", bufs=1, space="PSUM") as psumw,
        ):  # PSUM banks: 5 + 2 + 1 = 8
            # PE warmup: dummy matmuls release the HAM clock-gate (~3.4us of
            # sustained PE busy) while the first x block streams in.
            wsrc = warm.tile([128, 256], _F32, tag="wsrc")
            nc.gpsimd.memset(wsrc[:], 0.0)
            wv_w = wsrc[:].bitcast(_BF16)  # [128, 512]
            wv_x = wsrc[:].bitcast(_E3M4)  # [128, 1024]
            wps = psumw.tile([128, MT], _F32, tag="wps")
            for _ in range(N_WARMUP):
                nc.tensor.matmul(
                    wps[:], wv_w[:, :128], wv_x[:, :MT], start=True, stop=True
                )
            # weight loads ride the scalar HWDGE ring. Slot 1 (processed
            # first) is hoisted to kernel start; slot 0's loads are emitted
            # after the first block's chunk DMAs (its compute starts ~15us
            # in) so they don't delay the first x chunks on this ring.
            wts = [None] * S

            def load_weights(s):
                w1t = wpool.tile([128, DC, H], _BF16, tag="w1", name=f"w1t{s}")
                nc.scalar.dma_start(
                    w1t, w1[s].rearrange("p (dc h) -> p dc h", dc=DC)
                )
                b1t = wpool.tile([H, 1], _F32, tag="b1", name=f"b1t{s}")
                nc.scalar.dma_start(b1t, b1[s][:, None])
                w2t = wpool.tile([H, C], _BF16, tag="w2", name=f"w2t{s}")
                nc.scalar.dma_start(w2t, w2[s])
                b2t = wpool.tile([C, 1], _F32, tag="b2", name=f"b2t{s}")
                nc.scalar.dma_start(b2t, b2[s][:, None])
                wts[s] = (w1t, b1t, w2t, b2t)

            load_weights(1)
            load_weights(0)

            blist = _blist(M_slots)

            # Hybrid schedule per block:
            #   - first <=4 subtiles run WAVE-style (dc-outer over 4 psum
            #     banks) so the PE consumes each x chunk ~860ns after it
            #     lands instead of idling until chunk 7 of the block is in
            #     (chunk delivery ~1.6us each, a full block ~12us);
            #   - remaining subtiles run subtile-major (dc-inner into one
            #     bank, drain to ACT immediately).
            # Layer-2 matmuls + bias-adds are deferred on a queue and
            # flushed in PAIRS after later L1 groups: the ACT latency hides
            # under the next 8 matmuls, and back-to-back MM2s share the W2
            # stationary so only the first pays the weight-switch bubble.
            pendq = []  # (ht, ot, m0, mt, w2t, b2t, out_dma or None)

            def flush_one():
                ht, ot_p, m0, mt, w2t_p, b2t_p, out_args = pendq.pop(0)
                ps2 = psum2.tile([C, MT], _F32, tag="ps2")
                nc.tensor.matmul(
                    ps2[:, :mt], w2t_p, ht[:, :mt], start=True, stop=True
                )
                nc.vector.tensor_tensor(
                    ot_p[:, m0 : m0 + mt],
                    ps2[:, :mt],
                    b2t_p.to_broadcast([C, mt]),
                    mybir.AluOpType.add,
                )
                if out_args is not None:
                    eng, o0, ol, otb, src0 = out_args
                    eng.dma_start(outT[:, o0 : o0 + ol], otb[:, src0 : src0 + ol])

            blk_base = 0
            for bi, (s, x0, xl) in enumerate(blist):
                if wts[s] is None:
                    load_weights(s)
                w1t, b1t, w2t, b2t = wts[s]
                # per-d-chunk tiles/DMAs: each a single contiguous HBM span,
                # chunk-granular deps, all on the SP HWDGE ring. (Splitting
                # across rings measured WORSE: scalar serializes against
                # ACTIVATEs, gpsimd's software descriptor generation is too
                # slow for 128-descriptor reads.)
                xts = []
                for dc in range(DC):
                    xtc = xpool.tile([128, XB], _E3M4, tag=f"x{dc}", name=f"x_{dc}")
                    off = blk_base + dc * 128 * xl
                    nc.sync.dma_start(
                        xtc[:, :xl],
                        xAll[off : off + 128 * xl].rearrange("(p m) -> p m", p=128),
                    )
                    xts.append(xtc)
                blk_base += DC * 128 * xl
                ot = opool.tile([C, XB], _BF16, tag="o")
                last_block = bi == len(blist) - 1
                # final block: 256-col subtiles + per-subtile out-DMA on the
                # (by then idle) sync HWDGE ring -> short kernel tail
                subs = _chunks(xl, 256 if last_block else MT)

                def mk_out(j, m0, mt):
                    if last_block:
                        # alternate rings so the final per-subtile out-DMAs
                        # overlap instead of queueing on one ring
                        eng = nc.sync if j % 2 else nc.scalar
                        return (eng, x0 + m0, mt, ot, m0)
                    if j == len(subs) - 1:
                        # gpsimd (SWDGE): keeps the waiting out-DMA off the
                        # HWDGE rings so it can't block x-chunk DMAs
                        return (nc.gpsimd, x0, xl, ot, 0)
                    return None

                # final block runs subtile-major (wave=1): the first tail
                # subtile's L2+out-DMA then overlaps the second's compute
                wave = subs[:1] if last_block else subs[: min(4, len(subs))]
                rest = subs[len(wave) :]
                ps1s = [
                    psum1.tile([H, MT], _F32, tag="ps1", name=f"ps1_{j}")
                    for j in range(len(wave))
                ]
                for dc in range(DC):
                    for j, (m0, mt) in enumerate(wave):
                        nc.tensor.matmul(
                            ps1s[j][:, :mt],
                            w1t[:, dc, :],
                            xts[dc][:, m0 : m0 + mt],
                            start=(dc == 0),
                            stop=(dc == DC - 1),
                        )
                    # interleave the PREVIOUS block's deferred L2 pairs into
                    # this wave (one pair per two dc iterations): their ACTs
                    # get ~1.7us of runway, instead of all four bunching at
                    # the block boundary and stalling on the ScalarE chain
                    if dc % 2 == 1:
                        for _ in range(min(2, len(pendq))):
                            flush_one()
                for j, (m0, mt) in enumerate(wave):
                    ht = hpool.tile([H, MT], _BF16, tag="h")
                    nc.scalar.activation(ht[:, :mt], ps1s[j][:, :mt], relu, bias=b1t)
                    pendq.append((ht, ot, m0, mt, w2t, b2t, mk_out(j, m0, mt)))
                for j0, (m0, mt) in enumerate(rest):
                    j = len(wave) + j0
                    ps1 = psum1.tile([H, MT], _F32, tag="ps1")
                    for dc in range(DC):
                        nc.tensor.matmul(
                            ps1[:, :mt],
                            w1t[:, dc, :],
                            xts[dc][:, m0 : m0 + mt],
                            start=(dc == 0),
                            stop=(dc == DC - 1),
                        )
                    for _ in range(min(2, len(pendq))):
                        flush_one()
                    ht = hpool.tile([H, MT], _BF16, tag="h")
                    nc.scalar.activation(ht[:, :mt], ps1[:, :mt], relu, bias=b1t)
                    pendq.append((ht, ot, m0, mt, w2t, b2t, mk_out(j, m0, mt)))
            while pendq:
                flush_one()
    nc.compile()
    return nc


def _prepare(x, task_id, W1, b1, W2, b2, mm_dtype=MM_DTYPE):
    """Host-side routing + quantization.

    Returns (in_maps, meta) where meta = (slot_tasks, idx, counts, M_slots).
    slot_tasks[s][c] = task owned by core c's slot s.
    """
    np_bf16 = _np_bf16()
    np_e3 = _np_e3m4()
    x = np.ascontiguousarray(np.asarray(x, dtype=np.float32))
    task_id = np.asarray(task_id).astype(np.int64)
    W1 = np.asarray(W1, dtype=np.float32)
    b1 = np.asarray(b1, dtype=np.float32)
    W2 = np.asarray(W2, dtype=np.float32)
    b2 = np.asarray(b2, dtype=np.float32)

    order = np.argsort(task_id, kind="stable")
    counts = np.bincount(task_id, minlength=T)
    starts = np.concatenate([[0], np.cumsum(counts)])

    # rank tasks by count desc; core c gets rank c (slot 0) and rank
    # 15-c (slot 1) so each slot's pad target is its own worst case
    ranks = np.argsort(-counts, kind="stable")
    slot_tasks = [
        [int(ranks[c]) for c in range(N_CORES)],
        [int(ranks[T - 1 - c]) for c in range(N_CORES)],
    ]
    c128 = lambda n: max(128, int(-(-int(n) // 128) * 128))
    M_slots = (
        c128(counts[ranks[0]]),
        c128(counts[ranks[N_CORES]]),
    )

    # idx[s][c] = sample rows for that slot's task, padded with row 0
    idx = [np.zeros((N_CORES, M_slots[s]), dtype=np.int64) for s in range(S)]
    for s in range(S):
        for c in range(N_CORES):
            t = slot_tasks[s][c]
            idx[s][c, : counts[t]] = order[starts[t] : starts[t + 1]]

    xq = x.astype(np_e3)  # RNE quantization; |x| << 15.5 so no overflow
    w1b = W1.astype(np_bf16)
    w2b = W2.astype(np_bf16)

    blist = _blist(M_slots)
    in_maps = []
    for c in range(N_CORES):
        ts_c = [slot_tasks[s][c] for s in range(S)]
        rows = np.concatenate([idx[s][c] for s in range(S)])  # [M]
        xg = xq[rows]  # [M, D] e3m4
        # chunk-major [DC, 128, M]: row (dc, p) holds x[:, dc*128+p];
        # then packed per (block, chunk) so each [128, xl] DMA region is
        # one contiguous HBM span
        xT = xg.reshape(-1, DC, 128).transpose(1, 2, 0)
        xT = np.concatenate(
            [xT[:, :, x0 : x0 + xl].reshape(-1) for (_s, x0, xl) in blist]
        )
        # repack W1 [D, H] -> [128, DC*H] (partition-major, 2KB DMA rows)
        w1p = (
            w1b[ts_c]
            .reshape(S, DC, 128, H)
            .transpose(0, 2, 1, 3)
            .reshape(S, 128, DC * H)
        )
        in_maps.append(
            {
                "xAll": xT,
                "w1": np.ascontiguousarray(w1p),
                "b1": np.ascontiguousarray(b1[ts_c]),
                "w2": np.ascontiguousarray(w2b[ts_c]),
                "b2": np.ascontiguousarray(b2[ts_c]),
            }
        )
    return in_maps, (slot_tasks, idx, counts, M_slots)


def _unshard(results, meta, b_total=B):
    slot_tasks, idx, counts, M_slots = meta
    out = np.empty((b_total, C), dtype=np.float32)
    for c in range(N_CORES):
        yT = np.asarray(results[c]["outT"]).astype(np.float32)  # [C, M]
        off = 0
        for s in range(S):
            t = slot_tasks[s][c]
            cnt = counts[t]
            out[idx[s][c, :cnt]] = yT[:, off : off + cnt].T
            off += M_slots[s]
    return out


def kernel(x, task_id, W1, b1, W2, b2):
    in_maps, meta = _prepare(x, task_id, W1, b1, W2, b2)
    nc = _build(meta[3])
    try:
        res = run_bass_kernel_spmd(nc, in_maps, list(range(N_CORES)))
    except Exception:
        # transient NRT device hiccups (e.g. NRT_EXEC_UNIT_UNRECOVERABLE)
        # have been observed to succeed on retry
        res = run_bass_kernel_spmd(nc, in_maps, list(range(N_CORES)))
    return _unshard(res.results, meta, b_total=np.asarray(task_id).shape[0])
